# revision 102
# baseline (speedup 1.0000x reference)
"""FAVOR+ causal linear attention (relu feature map) on 8 Trainium2 NeuronCores.

Data-parallel over batch: B=8 -> one batch element per core. Per core, a
sequence-chunked scan over 16 chunks of C=128 positions, processed as 8
two-chunk blocks with an (M x V+1) bf16 running state in SBUF:

  phi = relu(x @ W) + eps
  out[l] = phi_q[l] @ (sum_{l'<=l} phi_k[l'] (x) v[l']) / (phi_q[l] . sum phi_k)

All matmuls run in bf16 (PSUM accumulation stays fp32; measured end-to-end
rel err ~4e-3 vs the 2e-2 gate), 4x PE throughput over fp32. Output is
staged bf16 and upcast on the host. phi_q drops its +eps (numerically
negligible; the denominator keeps phi_k's eps) so its relu can run on the
Act engine -- GPSIMD/Pool cannot touch PSUM on real HW, so every
PSUM-draining op must go to DVE (relu+eps, masks, recip, div, Sadd) or Act
(plain copies, pure Relu).

Per block (chunks c0,c1): A-triple (128,384) PSUM = [diag c0 | diag c1 |
cross c0->c1] in 3 matmuls; one DVE mask-multiply covers both diags, Act
copies the unmasked cross. phi_k in (C,M) orientation comes from 4 PE
transposes per QUAD into one PSUM bank + one Act copy. Per-block dS
matmuls close their PSUM accumulation group (groups cannot stay open
across engine reads) and a single DVE add folds them into the bf16 state.
Out chunks share a PSUM quad (128,260); one reciprocal (128,4) + one
3D-broadcast multiply (128,4,64) normalize 4 chunks at once; the last quad
normalizes pair-wise with one merged tail DMA so the final flush is small
and early. Emission is software-pipelined TWO blocks deep (front stage:
A/transposes/mask; back stage two iterations later: state + out matmuls),
so the in-order PE queue never waits on fresh vector results.

Host-side: values pre-scrambled in device layout (2,128,8*(V+1)) bf16 with
the ones-column baked in; k/q in device layout (128,1024) bf16; output in
device layout (2,128,8V) bf16, unscrambled + upcast on host. Input DMAs
issue from SP in criticality order (the HWDGE descriptor generator is one
shared ~630ns/issue device); constants and v ride Pool SWDGE in parallel.

Quirks worked around (this walrus/axon container): one sync-wait per
instruction (waits split onto NoOps post-lowering); PSUM banks must not mix
concurrent PE writes + engine reads on disjoint regions of one bank;
matmul operands must share a base partition; PSUM accumulation groups are
per-bank, strictly sequential, and unreadable while open.
"""

import numpy as np
import ml_dtypes

import concourse.bass as bass
import concourse.mybir as mybir
from concourse.tile import TileContext
from concourse.bass_utils import run_bass_kernel_spmd
from bass_rust import ScopedClock, VectorClock

f32 = mybir.dt.float32
bf16 = mybir.dt.bfloat16

B, D, L, M, V = 8, 64, 2048, 64, 64
KERNEL_EPS = 0.001
C = 128          # chunk length
NCH = L // C     # 16 chunks
NB = NCH // 2    # 8 two-chunk blocks
NCORES = 8

LABELS = {}      # instruction name -> semantic label (for sim profiling)


def _lab(label, bi):
    LABELS[bi.ins.name] = label
    return bi


# engine assignment for elementwise ops; entries: dve / act / pool
# (act only supports the plain copies: cpx, cpK, Scopy)
# NOTE: GPSIMD/Pool cannot access PSUM on real HW (compiler rejects it),
# so every PSUM-draining op must run on DVE or Act. Act additionally can
# only do copies, scale-activations and pure Relu (bias is pre-activation,
# so relu+eps is DVE-only). phi_q therefore drops its +eps (numerically
# negligible: denominator keeps phi_k's eps) so its relu can live on Act.
ENG = {
    # "dve": direct masked drain on DVE; "actpool": Act drains the raw A
    # pair to SBUF bf16, Pool (SBUF-only) applies the mask multiply.
    # (actpool measured slower end-to-end: the schedule is latency-bound.)
    "mask": ["dve"] * (NB - 1) + ["actdve"],
    "reluk": ["dve", "dve", "dve", "dve"],     # relu+eps: DVE only
    "reluq": ["act", "act", "act", "act"],     # pure relu on Act
    "div": ["dve"] * 4,                        # per quad
    "cpx": ["act"] * NB,
    "cpK": ["act"] * NB,
}


class _TileContextSplitDrain(TileContext):
    """This walrus build allows only ONE sync-wait command per instruction.
    Split the exit drain's waits into single-wait nops."""

    def _drain_and_barrier(self, tick_clock, wait_clock):
        from concourse.tile_scheduler import PROC_NAME_TO_IDX

        gc = tick_clock.global_clock
        ticks = list(gc)
        n = len(ticks)
        keep = set()
        for name, idx in PROC_NAME_TO_IDX.items():
            if name in ("PE", "DVE", "Activation", "SP", "Pool"):
                keep.add(idx)
        for inst in getattr(self.nc, "_tail_insts", []):
            p = inst.bass_scheduled_proc
            if p is not None:
                keep.add(p)
        for j in range(n):
            if ticks[j] <= 0 or j not in keep:
                continue
            vec = [0] * n
            vec[j] = ticks[j]
            nop = self.nc.sync.nop(nofuse=True, hint="split_drain_wait")
            wait_clock.add_sem_waits(nop.ins, ScopedClock({None: VectorClock(vec)}))
        self.nc.sync.drain()
        self.nc.all_engine_barrier()
        assert self.sems is not None
        popped = self.nc._tile_sem_poison_stack.pop()
        assert popped is self._sem_poison
        self.nc.clear_and_free_semaphores(list(self.sems.allocated().values()))
        self.nc.all_engine_barrier()


def _split_instruction_waits(nc):
    """Move excess sem waits (>1) onto same-engine NoOps inserted just before
    the instruction; the sequencer executes them in order, so semantics are
    unchanged."""
    counter = 0
    for f in nc.m.functions:
        for bb in f.blocks:
            il = list(bb.instructions)
            out = []
            changed = False
            for inst in il:
                si = inst.sync_info
                if si is not None and si.on_wait and len(si.on_wait) > 1:
                    waits = list(si.on_wait)
                    extra, keep = waits[:-1], waits[-1:]
                    for w in extra:
                        nop = mybir.InstNoOp(
                            name=f"waitsplit-{counter}", engine=inst.engine,
                            ins=[], outs=[],
                            sync_info=mybir.SyncInfo(on_wait=[w], on_update=[]))
                        counter += 1
                        out.append(nop)
                    si.on_wait = keep
                    inst.sync_info = si
                    changed = True
                out.append(inst)
            if changed:
                bb.instructions = out
    return counter


# iteration -> phi piece ids to emit (piece u covers phi cols 512u:512u+512
# of both phiK and phiQ, i.e. chunks 4u..4u+3 = blocks 2u, 2u+1)
PIECES_AT = {1: (1,), 2: (2,), 3: (3,)}


def build(repeats: int = 1, split_waits: bool = True) -> bass.Bass:
    LABELS.clear()
    nc = bass.Bass()
    keys_d = nc.dram_tensor("keys", [128, 1024], bf16, kind="ExternalInput")
    queries_d = nc.dram_tensor("queries", [128, 1024], bf16, kind="ExternalInput")
    valt_d = nc.dram_tensor("valt", [2, 128, 8 * (V + 1)], bf16, kind="ExternalInput")
    proj_d = nc.dram_tensor("proj", [128, M], bf16, kind="ExternalInput")
    # consts: cols 0:256 = causal mask pair [triu|triu], 256:320 = identity
    consts_d = nc.dram_tensor("consts", [128, 2 * C + 64], bf16, kind="ExternalInput")
    outt_d = nc.dram_tensor("outt", [2, 128, 8 * V], bf16, kind="ExternalOutput")

    mx = mybir.AluOpType.max
    ad = mybir.AluOpType.add
    ml = mybir.AluOpType.mult

    nc._tail_insts = []

    def E(kind, idx):
        return {"dve": nc.vector, "pool": nc.gpsimd}[ENG[kind][idx]]

    def CP(kind, idx, out, in_):
        e = ENG[kind][idx]
        if e == "act":
            return nc.scalar.copy(out, in_)
        if e == "dve":
            return nc.vector.tensor_copy(out, in_)
        return nc.gpsimd.tensor_copy(out, in_)

    with _TileContextSplitDrain(nc) as tc:
        with (
            tc.tile_pool(name="const", bufs=1) as const,
            tc.tile_pool(name="io", bufs=1) as io,
            tc.tile_pool(name="psPhi", bufs=2, space="PSUM") as psPhi,
            tc.tile_pool(name="psA", bufs=2, space="PSUM") as psA,
            tc.tile_pool(name="psK", bufs=1, space="PSUM") as psK,
            tc.tile_pool(name="psS", bufs=1, space="PSUM") as psS,
            tc.tile_pool(name="psO", bufs=2, space="PSUM") as psO,
            tc.tile_pool(name="sbA", bufs=3) as sbA,
            tc.tile_pool(name="sbK", bufs=2) as sbK,
            tc.tile_pool(name="sbS", bufs=2) as sbS,
            tc.tile_pool(name="sbR", bufs=2) as sbR,
            tc.tile_pool(name="sbT", bufs=2) as sbT,
        ):
            for _ in range(repeats):
                # ---- long-lived SBUF tensors
                w_s = const.tile([128, M], bf16, tag="w")
                ct_s = const.tile([128, 2 * C + 64], bf16, tag="ct")
                mk_s = ct_s[:, 0:256]
                id_s = ct_s[0:64, 256:320]
                k_s = io.tile([128, 1024], bf16, tag="k")
                q_s = io.tile([128, 1024], bf16, tag="q")
                vts = {}
                for h in range(2):
                    t = io.tile([128, 8 * (V + 1)], bf16, tag=f"v{h}")
                    vts[h] = t
                phiK = io.tile([64, 2048], bf16, tag="phiK")
                phiQ = io.tile([64, 2048], bf16, tag="phiQ")
                o_half = {}
                for h in range(2):
                    oh = io.tile([128, 8 * V], bf16, tag=f"out{h}")
                    o_half[h] = oh

                # ---- input DMAs. The HWDGE descriptor generator is ONE
                # shared device (~630ns per issue, serialized across SP/Act
                # queues); issuing everything from SP in criticality order
                # avoids arbitration races. Constants and v go via Pool
                # SWDGE which is independent.
                _lab("dma_w", nc.sync.dma_start(w_s[:], proj_d[:]))
                _lab("dma_k0a", nc.sync.dma_start(
                    k_s[0:64, 0:512], keys_d[0:64, 0:512]))
                _lab("dma_q0a", nc.sync.dma_start(
                    q_s[0:64, 0:512], queries_d[0:64, 0:512]))
                _lab("dma_ct", nc.gpsimd.dma_start(ct_s[:], consts_d[:]))
                _lab("dma_k0b", nc.sync.dma_start(
                    k_s[0:64, 512:1024], keys_d[0:64, 512:1024]))
                _lab("dma_q0b", nc.sync.dma_start(
                    q_s[0:64, 512:1024], queries_d[0:64, 512:1024]))
                _lab("dma_v0", nc.gpsimd.dma_start(vts[0][:], valt_d[0]))
                _lab("dma_k1", nc.sync.dma_start(
                    k_s[64:128, :], keys_d[64:128, :]))
                _lab("dma_q1", nc.sync.dma_start(
                    q_s[64:128, :], queries_d[64:128, :]))
                _lab("dma_v1", nc.gpsimd.dma_start(vts[1][:], valt_d[1]))



                def emit_phi_piece(u):
                    """phi cols 512u:512u+512. k and q each get their own
                    single-bank PSUM tile (double-buffered pool, so there is
                    no WAR stall between pieces). k: relu+eps on DVE;
                    q: pure Relu on Act. The startup piece is split into
                    256-col halves so block 0 gates on half the relu work."""
                    h, cc = u // 2, 512 * (u % 2)
                    rows = slice(64 * h, 64 * h + 64)
                    kcol = 512 * u
                    widths = ((0, 256), (256, 512)) if u == 0 else ((0, 512),)
                    ppk = psPhi.tile([64, 512], f32, tag="phi")
                    _lab(f"mm_phk{u}", nc.tensor.matmul(
                        ppk[:], lhsT=w_s[rows, :],
                        rhs=k_s[rows, cc:cc + 512], start=True, stop=True))
                    if u != 1:
                        for lo, hi in widths:
                            _lab(f"reluk{u}_{lo}", nc.vector.tensor_scalar(
                                phiK[:, kcol + lo:kcol + hi], ppk[:, lo:hi],
                                0.0, KERNEL_EPS, op0=mx, op1=ad))
                    else:
                        # DVE is the body bottleneck: drain raw via Act,
                        # relu+eps on Pool SBUF->SBUF (Pool cannot read
                        # PSUM, but bf16 SBUF is fine)
                        kraw = sbT.tile([64, 512], bf16, tag="kraw")
                        _lab(f"cpk{u}", nc.scalar.copy(kraw[:], ppk[:]))
                        _lab(f"reluk{u}", nc.gpsimd.tensor_scalar(
                            phiK[:, kcol:kcol + 512], kraw[:],
                            0.0, KERNEL_EPS, op0=mx, op1=ad))
                    ppq = psPhi.tile([64, 512], f32, tag="phi")
                    _lab(f"mm_phq{u}", nc.tensor.matmul(
                        ppq[:], lhsT=w_s[rows, :],
                        rhs=q_s[rows, cc:cc + 512], start=True, stop=True))
                    for j, (lo, hi) in enumerate(widths):
                        if (u == 0 and j == 1) or ENG["reluq"][u] == "dve":
                            _lab(f"reluq{u}_{lo}", nc.vector.tensor_scalar(
                                phiQ[:, kcol + lo:kcol + hi], ppq[:, lo:hi],
                                0.0, 0.0, op0=mx, op1=ad))
                        else:
                            _lab(f"reluq{u}_{lo}", nc.scalar.activation(
                                phiQ[:, kcol + lo:kcol + hi], ppq[:, lo:hi],
                                mybir.ActivationFunctionType.Relu))

                # per-iteration state carried between pipeline stages
                st = {}
                pend_epi = []
                S_bf_prev = None
                pOq = None

                for i in range(NB + 2):
                    # ---------- back stage: block b = i-2 state + outputs
                    # (two iterations behind, so the mask/Sadd/cpx results
                    # this stage consumes are comfortably old)
                    if i >= 2:
                        b = i - 2
                        c0, c1 = 2 * b, 2 * b + 1
                        h = b // 4
                        a0, a1 = c0 % 8, c1 % 8
                        g, qslot = b // 2, b % 2
                        At, pA_b, pk_q = st.pop(b)
                        pk_sb = pk_q[:, 128 * qslot:128 * qslot + 128]
                        pqT0 = phiQ[:, 128 * c0:128 * c0 + 128]
                        pqT1 = phiQ[:, 128 * c1:128 * c1 + 128]
                        Vt0 = vts[h][:, (V + 1) * a0:(V + 1) * a0 + V + 1]
                        Vt1 = vts[h][:, (V + 1) * a1:(V + 1) * a1 + V + 1]

                        # per-block dS in its own PSUM accumulation group,
                        # folded into the bf16 SBUF state by one DVE add
                        # (bf16 state accumulation measured numerically
                        # harmless). The last block's state is dead.
                        if b < NB - 1:
                            dS = psS.tile([M, V + 1], f32, tag="dS")
                            _lab(f"mm_s0_{b}", nc.tensor.matmul(
                                dS[:], lhsT=pk_sb[:, 0:64], rhs=Vt0,
                                start=True, stop=False))
                            _lab(f"mm_s1_{b}", nc.tensor.matmul(
                                dS[:], lhsT=pk_sb[:, 64:128], rhs=Vt1,
                                start=False, stop=True))
                            S_bf_new = sbS.tile([M, V + 1], bf16, tag="S")
                            if b == 0:
                                _lab(f"Sadd{b}", nc.vector.tensor_copy(
                                    S_bf_new[:], dS[:]))
                            else:
                                _lab(f"Sadd{b}", nc.vector.tensor_tensor(
                                    S_bf_new[:], dS[:], S_bf_prev[:], op=ad))
                        else:
                            S_bf_new = None

                        # out chunks into the PSUM quad (the last quad is
                        # two separate pair tiles so pair0's normalize/flush
                        # doesn't WAR-serialize against pair1's matmuls)
                        if g == 3:
                            pOq = psO.tile([C, 2 * (V + 1)], f32, tag="O")
                            cb0 = 0
                        elif qslot == 0:
                            pOq = psO.tile([C, 4 * (V + 1)], f32, tag="O")
                            cb0 = 0
                        else:
                            cb0 = 130
                        cb1 = cb0 + 65
                        # accumulation groups in one PSUM bank must be
                        # strictly sequential: finish chunk c0's group
                        # before opening c1's
                        if b > 0:
                            _lab(f"mm_i0_{b}", nc.tensor.matmul(
                                pOq[:, cb0:cb0 + 65], lhsT=pqT0,
                                rhs=S_bf_prev[:], start=True, stop=False))
                        _lab(f"mm_d0_{b}", nc.tensor.matmul(
                            pOq[:, cb0:cb0 + 65], lhsT=At[:, 0:128], rhs=Vt0,
                            start=(b == 0), stop=True))
                        if b > 0:
                            _lab(f"mm_i1_{b}", nc.tensor.matmul(
                                pOq[:, cb1:cb1 + 65], lhsT=pqT1,
                                rhs=S_bf_prev[:], start=True, stop=False))
                        _lab(f"mm_x1_{b}", nc.tensor.matmul(
                            pOq[:, cb1:cb1 + 65], lhsT=At[:, 256:384],
                            rhs=Vt0, start=(b == 0), stop=False))
                        _lab(f"mm_d1_{b}", nc.tensor.matmul(
                            pOq[:, cb1:cb1 + 65], lhsT=At[:, 128:256],
                            rhs=Vt1, start=False, stop=True))
                        if S_bf_new is not None:
                            S_bf_prev = S_bf_new

                        # epilogue: normalize + flush. Quad-granular (4
                        # chunks in one recip+div+DMA), except the LAST quad
                        # which goes pair-wise so the final DMA is small and
                        # fires as early as possible.
                        def _norm_flush(tag, pcols, ncols, ocols, dcols, n,
                                        dma=True, pOq_=None, h_=None,
                                        two_step=False):
                            pOq_ = pOq if pOq_ is None else pOq_
                            h_ = h if h_ is None else h_
                            rec = sbR.tile([C, n], f32, tag="rec")
                            _lab(f"recip{tag}", nc.vector.reciprocal(
                                rec[:], pOq_[:, ncols]))
                            ostage = o_half[h_][:, ocols]
                            pOv = pOq_[:, pcols].rearrange(
                                "p (c v) -> p c v", c=n)[:, :, 0:64]
                            recb = rec[:].unsqueeze(2).broadcast_to([C, n, 64])
                            osv = ostage.rearrange("p (c v) -> p c v", c=n)
                            if two_step:
                                # Act drains raw into staging; idle-in-body
                                # Pool multiplies by the recips in place
                                _lab(f"divcp{tag}", nc.scalar.copy(osv, pOv))
                                _lab(f"div{tag}", nc.gpsimd.tensor_tensor(
                                    osv, osv, recb, op=ml))
                            else:
                                _lab(f"div{tag}", nc.vector.tensor_tensor(
                                    osv, pOv, recb, op=ml))
                            if not dma:
                                return
                            di = _lab(f"dma_o{tag}", nc.sync.dma_start(
                                outt_d[h_, :, dcols], ostage))
                            nc._tail_insts.append(di.ins)

                        if g == 3:
                            # pair-wise for the last quad (own pair tile),
                            # emitted immediately: it IS the tail. One
                            # merged DMA after the second pair's divide.
                            ob = 256 * (g % 2) + 128 * qslot
                            _norm_flush(f"{g}{qslot}",
                                        slice(0, 130),
                                        slice(64, 130, 65),
                                        slice(ob, ob + 128),
                                        slice(ob, ob + 128), 2, dma=False)
                            if qslot == 1:
                                di = _lab("dma_o3", nc.sync.dma_start(
                                    outt_d[1, :, 256:512],
                                    o_half[1][:, 256:512]))
                                nc._tail_insts.append(di.ins)
                        elif qslot == 1 and g == 0:
                            # h=0: defer the DMA; quad 1's flush sends the
                            # whole half in one transfer. Epilogues of quads
                            # 0-2 are deferred one iteration so their divs
                            # don't sit ahead of the next Sadd in the DVE
                            # queue.
                            pend_epi.append(
                                lambda p=pOq, hh=h: _norm_flush(
                                    "0", slice(0, 260), slice(64, 260, 65),
                                    slice(0, 256), None, 4, dma=False,
                                    pOq_=p, h_=hh, two_step=True))
                        elif qslot == 1 and g == 1:
                            def _fl1(p=pOq, hh=h):
                                _norm_flush("1", slice(0, 260),
                                            slice(64, 260, 65),
                                            slice(256, 512), None, 4,
                                            dma=False, pOq_=p, h_=hh,
                                            two_step=True)
                                di = _lab("dma_o01", nc.sync.dma_start(
                                    outt_d[0, :, 0:512],
                                    o_half[0][:, 0:512]))
                                nc._tail_insts.append(di.ins)
                            pend_epi.append(_fl1)
                        elif qslot == 1:
                            pend_epi.append(
                                lambda p=pOq, hh=h, gg=g: _norm_flush(
                                    f"{gg}", slice(0, 260),
                                    slice(64, 260, 65),
                                    slice(256 * (gg % 2), 256 * (gg % 2) + 256),
                                    slice(256 * (gg % 2), 256 * (gg % 2) + 256),
                                    4, pOq_=p, h_=hh))

                        for f_ in pend_epi:
                            f_()
                        pend_epi.clear()

                    # ---------- front stage: block i inputs
                    if i < NB:
                        b = i
                        if i == 0:
                            # piece 0 feeds THIS block: must precede its
                            # A matmuls in program order (dep tracking)
                            emit_phi_piece(0)
                        c0, c1 = 2 * b, 2 * b + 1
                        pkT0 = phiK[:, 128 * c0:128 * c0 + 128]
                        pkT1 = phiK[:, 128 * c1:128 * c1 + 128]
                        pqT0 = phiQ[:, 128 * c0:128 * c0 + 128]
                        pqT1 = phiQ[:, 128 * c1:128 * c1 + 128]

                        # A-triple: [diag c0 | diag c1 | cross c0->c1]
                        pA = psA.tile([C, 3 * C], f32, tag="A")
                        _lab(f"mm_A0_{b}", nc.tensor.matmul(
                            pA[:, 0:128], lhsT=pkT0, rhs=pqT0,
                            start=True, stop=True))
                        _lab(f"mm_A1_{b}", nc.tensor.matmul(
                            pA[:, 128:256], lhsT=pkT1, rhs=pqT1,
                            start=True, stop=True))
                        _lab(f"mm_Ax_{b}", nc.tensor.matmul(
                            pA[:, 256:384], lhsT=pkT0, rhs=pqT1,
                            start=True, stop=True))
                        # At even blocks: ALL FOUR phi_k transposes of the
                        # quad (both blocks come from the same phi piece) +
                        # the quad copy to bf16 SBUF immediately. This keeps
                        # the cpK hop a full iteration ahead of the state
                        # matmuls that consume it.
                        if b % 2 == 0:
                            pKq = psK.tile([C, 4 * M], bf16, tag="K")
                            pkT2 = phiK[:, 128 * (c0 + 2):128 * (c0 + 2) + 128]
                            pkT3 = phiK[:, 128 * (c0 + 3):128 * (c0 + 3) + 128]
                            for j, src in enumerate((pkT0, pkT1, pkT2, pkT3)):
                                _lab(f"trK{j}_{b}", nc.tensor.transpose(
                                    pKq[:, 64 * j:64 * j + 64], src, id_s))
                            pk_sbq = sbK.tile([C, 4 * M], bf16, tag="pk")
                            _lab(f"cpK{b // 2}", CP("cpK", b, pk_sbq[:],
                                                    pKq[:]))
                        # phi pieces after this block's matmuls: these
                        # pieces have 1+ block of lookahead, the A matmuls
                        # of this block do not
                        for u in PIECES_AT.get(i, ()):
                            emit_phi_piece(u)
                        # mask both diags in one op (cross is copied after
                        # the back stage -- it is less urgent than Sadd)
                        At = sbA.tile([C, 3 * C], bf16, tag="At")
                        if ENG["mask"][b] == "dve":
                            _lab(f"mask{b}", nc.vector.tensor_tensor(
                                At[:, 0:256], pA[:, 0:256], mk_s, op=ml))
                        else:
                            # Act drains raw; the bf16 all-SBUF multiply
                            # runs at 2x on DVE ("actdve") or on Pool
                            Araw = sbA.tile([C, 2 * C], bf16, tag="Araw")
                            _lab(f"maskcp{b}", nc.scalar.copy(
                                Araw[:], pA[:, 0:256]))
                            e2 = (nc.vector if ENG["mask"][b] == "actdve"
                                  else nc.gpsimd)
                            _lab(f"mask{b}", e2.tensor_tensor(
                                At[:, 0:256], Araw[:], mk_s, op=ml))
                        st[b] = (At, pA, pk_sbq)

                    # cross copy for the front-stage block, deferred past
                    # the back stage so Act serves the state chain first
                    if i < NB:
                        Atf, pAf, _pkq = st[i]
                        _lab(f"cpx{i}", CP("cpx", i, Atf[:, 256:384],
                                           pAf[:, 256:384]))

    if split_waits:
        _split_instruction_waits(nc)
    return nc


_CONSTS = None


def _consts():
    global _CONSTS
    if _CONSTS is None:
        tri = np.triu(np.ones((C, C), dtype=np.float32))
        ct = np.zeros((128, 2 * C + 64), dtype=np.float32)
        ct[:, 0:128] = tri
        ct[:, 128:256] = tri
        ct[0:64, 256:320] = np.eye(64, dtype=np.float32)
        _CONSTS = ct.astype(ml_dtypes.bfloat16)
    return _CONSTS


def kernel(keys, values, queries, proj_matrix):
    bf = ml_dtypes.bfloat16
    # device layout (128, 1024): rows 0:64 = D dims for positions 0:1024,
    # rows 64:128 = positions 1024:2048
    kd = np.asarray(keys, np.float32).reshape(B, D, 2, 1024)
    kd = np.ascontiguousarray(kd.transpose(0, 2, 1, 3).reshape(B, 128, 1024))
    qd = np.asarray(queries, np.float32).reshape(B, D, 2, 1024)
    qd = np.ascontiguousarray(qd.transpose(0, 2, 1, 3).reshape(B, 128, 1024))
    vT = np.asarray(values, np.float32).transpose(0, 2, 1)  # (B, L, V)
    vT = vT.reshape(B, 2, 8, 128, V).transpose(0, 1, 3, 2, 4)  # (B,2,128,8,V)
    valt = np.ones((B, 2, 128, 8, V + 1), dtype=np.float32)
    valt[..., 0:V] = vT
    valt = np.ascontiguousarray(valt.reshape(B, 2, 128, 8 * (V + 1)))
    pm = np.asarray(proj_matrix, np.float32)
    pmd = np.ascontiguousarray(np.concatenate([pm, pm], axis=0))  # (128, M)
    consts = _consts()

    nc = build()
    in_maps = [
        {
            "keys": kd[b].astype(bf), "valt": valt[b].astype(bf),
            "queries": qd[b].astype(bf), "proj": pmd.astype(bf),
            "consts": consts,
        }
        for b in range(B)
    ]
    res = run_bass_kernel_spmd(nc, in_maps, list(range(NCORES)))
    # outt: (2, 128, 8*V) device layout -> out (V, L): out[v, 1024h+128a+p]
    outs = []
    for b in range(B):
        ot = np.asarray(res.results[b]["outt"]).astype(np.float32)
        ot = ot.reshape(2, 128, 8, V)
        outs.append(ot.transpose(3, 0, 2, 1).reshape(V, L))
    return np.ascontiguousarray(np.stack(outs, axis=0), dtype=np.float32)


if __name__ == "__main__":
    rng = np.random.default_rng(0)
    ks = rng.standard_normal((B, D, L), dtype=np.float32)
    vs = rng.standard_normal((B, V, L), dtype=np.float32)
    qs = rng.standard_normal((B, D, L), dtype=np.float32)
    pm = np.linalg.qr(rng.standard_normal((D, M)))[0].astype(np.float32)
    o = kernel(ks, vs, qs, pm)
    print("kernel output", o.shape, o.dtype)
